# revision 53
# baseline (speedup 1.0000x reference)
"""CompressKV gating kernel for 8 Trainium2 NeuronCores.

Reference computation (per batch b, head h):
    x_s = x[b, :, h, :]                                  # [N=4096, D=128]
    windows n = 0..254, rows r = 16n + k, k = 0..31
    logits[n, g] = sum_{k,d} x_s[16n+k, d] * W[g, k, d]  # W = W_gate.reshape(32,32,128)
    gate = softmax_g(logits)
    out[n, d] = sum_k gate[n, k] * x_s[16n+k, d]

Sharding: B*H = 32 (b,h) slices, 4 per core, no cross-core communication.
Host pre-packs x per core in two bf16 layouts (the kernel is input-DMA
bound at ~8.4 MiB/core; everything else hides under the input stream):
  xt: d-major (transposed)   [4, 128(d), 4112(r pad)]
  xn: chunk-major native     [4, 128(p), 32(c)*128(d)]  (chunk c = rows 128c+p)
plus the gate weight transposed to d-major wt[d, k*32+g].

On-device pipeline per slice (PE-stream order = DMA arrival order):
  A) logits[g, n] via 32 accumulating matmuls into ONE psum tile [32, 256]
     (no col-tiling, no DVE fold).
  B) e = exp(logits) (ACT, reads psum); den via ones-matmul -> [1, 256];
     rden = reciprocal (DVE); rden broadcast to 32 partitions via a K=1
     ones-matmul; gate = e * rden (DVE).  Gates are normalized BEFORE
     pooling so the output needs no per-window scaling.
  C) banded S matrix from gate via 9 band-matrix matmuls (psS[j, c] layout),
     then one DVE scatter-copy into zero-padded per-chunk buffers
     B_pad[:, 64c+31+j] so that pooling lhsT slices are 32-aligned.
  D) pooling: out[w, d] for 32-window groups t: psum tile partition base
     32(t%4); 5 accumulating matmuls per group (chunks 4t..4t+4), lhsT =
     B_pad[:, 64c+32-8i : 64c+64-8i], rhs = xn chunk.  Output is directly
     [windows, d] -- no transposes.  DVE copy psum->bf16, DMA out on the
     DVE HWDGE ring (outputs must not queue behind inputs on the SP ring).
"""

import sys

import numpy as np

for _p in ("/opt/trn_rl_repo", "/opt/pypackages"):
    if _p not in sys.path:
        sys.path.append(_p)

import ml_dtypes

_B, _N, _H, _D = 2, 4096, 16, 128
_K = 32          # window (kernel) size
_ST = 16         # stride
_NB = 255        # num windows
_NC = 8          # cores
_SL = 4          # (b,h) slices per core
_NT = 4112       # padded r extent for xt (>= 16*255 + 31 + 1)
_NCH = 32        # 128-row chunks per slice

_prog_cache = {}


def _build_program():
    import concourse.mybir as mybir
    from concourse import bacc, masks, tile

    f32 = mybir.dt.float32
    bf16 = mybir.dt.bfloat16

    nc = bacc.Bacc()
    xt = nc.dram_tensor("xt", [_SL, 128, _NT], bf16, kind="ExternalInput")
    xn = nc.dram_tensor("xn", [_SL, 128, _NCH * _D], bf16, kind="ExternalInput")
    wt = nc.dram_tensor("wt", [128, _K * _K], bf16, kind="ExternalInput")
    out = nc.dram_tensor("out", [_SL, 256, _D], bf16, kind="ExternalOutput")

    with tile.TileContext(nc) as tc:
        with (
            tc.tile_pool(name="const", bufs=1) as cpool,
            tc.tile_pool(name="data", bufs=4) as dpool,
            tc.tile_pool(name="small", bufs=2) as spool,
            tc.tile_pool(name="psA", bufs=1, space="PSUM") as psa_pool,
            tc.tile_pool(name="psL", bufs=1, space="PSUM") as psl_pool,
            tc.tile_pool(name="psMT", bufs=1, space="PSUM") as psmt_pool,
            tc.tile_pool(name="psS", bufs=1, space="PSUM") as pss_pool,
            tc.tile_pool(name="psO0", bufs=1, space="PSUM") as pso_pool0,
            tc.tile_pool(name="psO1", bufs=1, space="PSUM") as pso_pool1,
            tc.tile_pool(name="psO2", bufs=1, space="PSUM") as pso_pool2,
            tc.tile_pool(name="psO3", bufs=1, space="PSUM") as pso_pool3,
        ):
            # ---- constants / scratch ----
            # gpsimd order matters: ident/mband/fstack first (fstack gates
            # the PE warm-up junk), bpad memsets after
            ones32 = cpool.tile([32, 1], bf16)
            nc.vector.memset(ones32[:], 1.0)
            ident = cpool.tile([128, 128], f32)
            masks.make_identity(nc, ident[:])

            # band master: mband[k, c] = 1 iff c == k + 128.  Slicing cols
            # [144-16j : 272-16j] gives the [32k, 128r] band matrix with
            # 1 at r == k + 16j - 16 (rows outside [0,128) auto-dropped).
            mband = cpool.tile([32, 272], bf16)
            nc.gpsimd.memset(mband[:], 0.0)
            nc.gpsimd.affine_select(
                out=mband[:],
                in_=mband[:],
                compare_op=mybir.AluOpType.not_equal,
                fill=1.0,
                base=128,
                pattern=[[-1, 272]],
                channel_multiplier=1,
            )
            # fold matrix: fstack[p, g] = 1 iff p % 32 == g (4 stacked
            # 32x32 identities) -- folds the 4 col-tile groups of psA in
            # a single K=128 matmul.  Built early: it also gates the PE
            # warm-up junk matmuls.
            fstack = cpool.tile([128, 32], bf16)
            nc.gpsimd.memset(fstack[:], 0.0)
            for j in range(4):
                nc.gpsimd.affine_select(
                    out=fstack[32 * j : 32 * j + 32, :],
                    in_=fstack[32 * j : 32 * j + 32, :],
                    compare_op=mybir.AluOpType.not_equal,
                    fill=1.0,
                    base=0,
                    pattern=[[-1, 32]],
                    channel_multiplier=1,
                )

            # zero-padded per-chunk S buffers, [128, 32 chunks x 64 cols];
            # only cols 64c+31 .. 64c+39 are ever (re)written per slice, the
            # zeros make the 32-aligned pooling lhsT slices correct.
            bpad = []
            for i in range(_SL):
                b3 = cpool.tile([128, _NCH, 64], bf16, name=f"bpad{i}")
                (nc.vector if i % 2 == 0 else nc.gpsimd).memset(b3[:], 0.0)
                bpad.append(b3)

            # weight on the ACT ring so it doesn't delay xt0 on the SP ring
            wt_sb = cpool.tile([128, _K * _K], bf16)
            nc.scalar.dma_start(wt_sb[:], wt[:, :])

            # ---- input DMAs, in consumption-paced order, all on SP ring ----
            # xt ships as two half-tiles (residues 0-7 / 8-15) so stage A
            # starts as soon as the first half lands (tile-granular deps)
            xt_sb = [
                [
                    dpool.tile([128, _NT // 2], bf16, tag=f"xt{h}", name=f"xt{s}_{h}")
                    for h in range(2)
                ]
                for s in range(_SL)
            ]
            xn_sb = [
                dpool.tile([128, _NCH * _D], bf16, tag="xn", name=f"xn{s}")
                for s in range(_SL)
            ]
            # all xt's first: every slice's logits/S chain completes while
            # the xn stream is still arriving; pools are then pure consumers
            # paced by xn arrival, so the post-stream tail is just the last
            # pool group + one output DMA
            for s in range(_SL):
                for h in range(2):
                    nc.sync.dma_start(
                        xt_sb[s][h][:], xt[s, :, 2056 * h : 2056 * h + 2056]
                    )
            for s in range(_SL):
                nc.sync.dma_start(xn_sb[s][:], xn[s, :, :])

            # PE warm-up: ~2us of junk matmuls gated on the early gpsimd
            # fstack const keep the HAM activity window busy so stage A
            # starts at 2.4 GHz instead of 1.2 (the throttle releases only
            # after ~3.4us of sustained PE activity)
            psJ = psmt_pool.tile([128, 258], f32, tag="m")
            for w in range(40):
                nc.tensor.matmul(
                    psJ[0:32, 0:32],
                    fstack[:, 0:32],
                    fstack[:, 0:32],
                    start=True,
                    stop=True,
                    skip_group_check=True,
                )

            state = {}

            def emit_a(s):
                # logits, 4-way col-tiled: group q loads 4 weight sets
                # (k = 4q+j at array cols 32j) -- the rotating col groups
                # let each LDW overlap the previous group's streams.
                # psA[32j+g, n] accumulates over q.  xt is residue-grouped
                # on host (xtg[d, 257m+o] = x[16o+m, d]) so every rhs
                # stream is CONTIGUOUS (strided bf16 streams run ~5x slow).
                psA = psa_pool.tile([128, 256], f32, tag="psA", name=f"psA_{s}")
                ks = (
                    list(range(0, 8)) + list(range(16, 24))      # xt half 0
                    + list(range(8, 16)) + list(range(24, 32))   # xt half 1
                )
                for q in range(8):
                    for j in range(4):
                        k = ks[4 * q + j]
                        m, o = k % 16, k // 16
                        nc.tensor.matmul(
                            psA[32 * j : 32 * j + 32, :],
                            wt_sb[:, 32 * k : 32 * k + 32],
                            xt_sb[s][m // 8][
                                :, 257 * (m % 8) + o : 257 * (m % 8) + o + 256
                            ],
                            start=(q == 0),
                            stop=(q == 7),
                            tile_position=(0, 32 * j),
                            skip_group_check=True,
                        )
                state[s] = psA

            def emit_exp(s):
                # fold the 4 col groups with one K=128 matmul, then exp.
                # Emitted early so the ACT stream is exp0, exp1, dma0, exp2,
                # dma1, ... -- exp_{s+1} must not queue behind slice s's
                # output-DMA waits on the in-order ACT sequencer.
                psA = state.pop(s)
                asb = spool.tile([128, 256], bf16, tag="asb", name=f"asb_{s}")
                nc.vector.tensor_copy(asb[:], psA[:])
                psL = psl_pool.tile([32, 256], f32, tag="psL")
                nc.tensor.matmul(
                    psL[:, :], fstack[:, :], asb[:, :], skip_group_check=True
                )
                e_kn = spool.tile([32, 256], bf16, tag="e", name=f"e_{s}")
                nc.scalar.activation(
                    e_kn[:], psL[:], mybir.ActivationFunctionType.Exp
                )
                state[s] = e_kn

            def emit_b(s):
                e_kn = state.pop(s)
                # ---- S matrix FIRST (gates the pool path): ----
                #      psS[r, j, c] = e[r-16j+16, 8c-1+j]
                psS = pss_pool.tile([128, 9, 32], f32, tag="psS")
                for j in range(9):
                    c0 = 1 if j == 0 else 0
                    c1 = 31 if j == 8 else 32
                    nc.tensor.matmul(
                        psS[:, j, c0:c1],
                        mband[:, 144 - 16 * j : 272 - 16 * j],
                        e_kn[:, 8 * c0 + j - 1 : 8 * (c1 - 1) + j : 8],
                        start=True,
                        stop=True,
                        skip_group_check=True,
                    )
                # scatter into the padded layout: bpad[:, c, 31+j] = psS[:, j, c]
                nc.vector.tensor_copy(
                    bpad[s][:, :, 31:40],
                    psS[:].rearrange("p j c -> p c j"),
                )

                # den path feeds only the copy-out -- after the S build.
                # den[n] = sum_k e[k, n] -> [1, 256]; transpose to partitions
                # (2 PE transposes) and reciprocal on [128, 2] (DVE).
                psMT = psmt_pool.tile([128, 258], f32, tag="m")
                nc.tensor.matmul(
                    psMT[0:1, 0:256], ones32[:, 0:1], e_kn[:, :],
                    skip_group_check=True,
                )
                den_sb = spool.tile([1, 256], f32, tag="d")
                nc.vector.tensor_copy(den_sb[:], psMT[0:1, 0:256])
                nc.tensor.transpose(
                    psMT[0:128, 256:257], den_sb[0:1, 0:128], ident[0:1, 0:1]
                )
                nc.tensor.transpose(
                    psMT[0:127, 257:258], den_sb[0:1, 128:255], ident[0:1, 0:1]
                )
                rden = dpool.tile([128, 2], f32, tag="r")
                nc.vector.reciprocal(rden[0:128, 0:1], psMT[0:128, 256:257])
                nc.vector.reciprocal(rden[0:127, 1:2], psMT[0:127, 257:258])
                state[s] = rden

            pso_pools = [pso_pool0, pso_pool1, pso_pool2, pso_pool3]

            def emit_pool_memset(s):
                # interleaved pool groups can't use start=True (the
                # has_written clear is coarser than the addressed region);
                # memset instead, emitted ahead so pools aren't serialized
                # behind earlier slices' copyouts in the DVE queue.
                # One POOL per slice: psum access tracking is tensor-level,
                # so sharing a pool would serialize pool_{s+1}'s PE writes
                # behind co_s's DVE reads.
                psO = pso_pools[s].tile([128, 256], f32, tag="o", name=f"psO_{s}")
                nc.vector.memset(psO[:], 0.0)
                state[("o", s)] = psO

            def emit_pool(s):
                b3 = bpad[s]
                psO = state.pop(("o", s))
                # i-outer / t-inner: consecutive MMs rotate PE col groups
                # (t%4) so each LDWEIGHTS hides under the previous stream
                for i in range(5):
                    for t in range(8):
                        c = 4 * t + i
                        cmax = min(4 * t + 4, _NCH - 1)
                        if c > cmax:
                            continue
                        h0 = 128 * (t // 4)
                        nc.tensor.matmul(
                            psO[32 * (t % 4) : 32 * (t % 4) + 32, h0 : h0 + 128],
                            b3[:, c, 32 - 8 * i : 64 - 8 * i],
                            xn_sb[s][:, 128 * c : 128 * c + 128],
                            start=False,
                            stop=(c == cmax),
                            tile_position=(0, 32 * (t % 4)),
                            skip_group_check=True,
                        )
                rden = state.pop(s)
                # single staging tile + ONE output DMA per slice: each HWDGE
                # config costs ~700-1100ns serialized on the ACT sequencer,
                # so fewer/bigger output DMAs shorten the tail.  Dram row 255
                # is garbage (window 255 doesn't exist); host drops it.
                osb = dpool.tile([128, 256], bf16, tag="o", name=f"o_{s}")
                nc.vector.tensor_scalar(
                    osb[:, 0:128], psO[0:128, 0:128], rden[0:128, 0:1], None,
                    mybir.AluOpType.mult,
                )
                nc.vector.tensor_scalar(
                    osb[:, 128:256], psO[0:128, 128:256], rden[0:128, 1:2], None,
                    mybir.AluOpType.mult,
                )
                # outputs go on the ACT HWDGE ring: the SP ring's queue is
                # busy with input descriptors until the very end
                nc.scalar.dma_start(
                    out[s].rearrange("(h p) d -> p h d", h=2), osb[:]
                )

            emit_a(0)
            emit_exp(0)
            emit_a(1)
            for s in range(_SL):
                if s + 1 < _SL:
                    emit_exp(s + 1)
                emit_b(s)
                if s + 2 < _SL:
                    emit_a(s + 2)
            for s in range(_SL):
                emit_pool_memset(s)
            for s in range(_SL):
                emit_pool(s)

    nc.compile()
    return nc


def _get_program():
    if "nc" not in _prog_cache:
        _prog_cache["nc"] = _build_program()
    return _prog_cache["nc"]


def _host_inputs(x, W_gate):
    bf16 = ml_dtypes.bfloat16
    x = np.asarray(x, dtype=np.float32)
    W = np.asarray(W_gate, dtype=np.float32)
    # wt[d, k*32+g] = W_gate[g, k*128+d]
    wt_host = np.ascontiguousarray(
        W.reshape(_K, _K, _D).transpose(2, 1, 0).reshape(_D, _K * _K)
    ).astype(bf16)
    in_maps = []
    xpad = np.zeros((_NT, _D), dtype=np.float32)
    for core in range(_NC):
        xn = np.empty((_SL, 128, _NCH * _D), dtype=bf16)
        xt = np.empty((_SL, 128, _NT), dtype=bf16)
        for si in range(_SL):
            p = core * _SL + si
            b, h = p // _H, p % _H
            xs = x[b, :, h, :]  # [4096, 128]
            xn[si] = (
                xs.reshape(_NCH, 128, _D).transpose(1, 0, 2).reshape(128, _NCH * _D)
            ).astype(bf16)
            # residue-grouped transpose: xt[d, 257m+j] = x[16j+m, d]
            xpad[:_N] = xs
            xt[si] = (
                xpad.reshape(257, 16, _D).transpose(2, 1, 0).reshape(_D, _NT)
            ).astype(bf16)
        in_maps.append({"xn": xn, "xt": xt, "wt": wt_host})
    return in_maps


def _assemble(results):
    out = np.empty((_B, _NB, _H, _D), dtype=np.float32)
    for core in range(_NC):
        o = np.asarray(results[core]["out"]).astype(np.float32)
        for si in range(_SL):
            p = core * _SL + si
            out[p // _H, :, p % _H, :] = o[si, :_NB]
    return out


def _install_trace_hooks():
    """Shim the axon NTFF profile hook (missing in this image) so
    run_bass_kernel_spmd(trace=True) can collect a HW profile, and neuter
    the artifact upload (zero-egress container)."""
    import contextlib
    import ctypes
    import types

    try:
        from antenv.axon_hooks import get_axon_ntff_profile_hook  # noqa: F401

        return
    except ImportError:
        pass

    lib = ctypes.CDLL("/opt/axon/libaxon_pjrt.so")
    if not hasattr(lib, "axon_start_nrt_profile"):
        return
    lib.axon_start_nrt_profile.argtypes = [
        ctypes.POINTER(ctypes.c_int64),
        ctypes.c_size_t,
    ]
    lib.axon_start_nrt_profile.restype = ctypes.c_int64
    lib.axon_stop_nrt_profile.argtypes = [ctypes.c_char_p]
    lib.axon_stop_nrt_profile.restype = ctypes.c_int64

    @contextlib.contextmanager
    def _hook(output_dir, device_ids):
        import jax

        jax.devices()
        if device_ids:
            ids = (ctypes.c_int64 * len(device_ids))(*device_ids)
            rc = lib.axon_start_nrt_profile(ids, len(device_ids))
        else:
            rc = lib.axon_start_nrt_profile(None, 0)
        if rc != 0:
            raise RuntimeError(f"axon_start_nrt_profile rc={rc}")
        try:
            yield
        finally:
            n = lib.axon_stop_nrt_profile(str(output_dir).encode())
            print(f"profile: {n} file(s) written to {output_dir}")

    mod = types.ModuleType("antenv.axon_hooks")
    mod.get_axon_ntff_profile_hook = lambda: _hook
    mod.set_axon_ntff_profile_hook = lambda h: None
    sys.modules["antenv.axon_hooks"] = mod

    from concourse import bass_utils as bu

    bu.upload_artifacts = lambda tmpdir: tmpdir


def run(x, W_gate, trace=False, **kw):
    from concourse.bass_utils import run_bass_kernel_spmd

    if trace:
        _install_trace_hooks()
    nc = _get_program()
    in_maps = _host_inputs(x, W_gate)
    res = run_bass_kernel_spmd(nc, in_maps, list(range(_NC)), trace=trace, **kw)
    return _assemble(res.results), res


def kernel(x, W_gate):
    out, _ = run(x, W_gate)
    return out
